# revision 4
# baseline (speedup 1.0000x reference)
"""Causal single-head attention (B=4, S=2048, d=1024) on 8 TRN2 NeuronCores.

Sharding: core c -> batch b = c//2, subset s = c%2. Per batch the 16
query blocks (128 rows) are assigned in balanced causal pairs: core
(b,s) owns pairs (lo_i, hi_i) = (2i+s, 15-2i-s), i=0..3, giving every
core 68 true causal score tiles (padded to a uniform 72). Every core
runs the identical instruction stream; causal boundaries come from
per-core 0/1 mask tiles supplied as input data.

K/V projections are tensor-parallel within each core pair: core (b,s)
computes the d_out-half s of v (bf16) and kT (fp8, x32-scaled) for the
whole batch; halves are exchanged with pairwise AllGathers
([[0,1],[2,3],[4,5],[6,7]]). V runs first so its (2x larger) gather
lands long before Phase B needs it.

Precision: projections and AV run in bf16 (fp32 PSUM). Scores run in
fp8e4m3 DoubleRow (256-deep contraction per pass = 2x bf16 FLOPs):
host folds x32 into Wq and Wk so q' = 32q, k' = 32k sit in the fp8
sweet spot; the combined 2^15 scale is removed inside the EXP
activation (exp(s' / 32768)).

Compute:
  P2: v half -> vg_in (bf16), AllGather -> vv [2048, d]
  P1: kT half (bf16 matmul) -> fp8 kg_in, AllGather -> kt8 [d, 2048]
  P0: qT' -> qt8 [d, 1024] fp8 (overlaps the gathers)
  A:  kb-major over score tiles; per (kb, dc-pair j) the kt8 weight
      slice is reused across every pair i whose range covers kb
      (shared [k128,q256] tiles for kb < 2i+2, solo [k128,q128] for
      kb in [2i+2, 16-2i)), with up to 4 interleaved PSUM groups.
      p = exp(s'/32768) -> bf16, masked at causal boundary tiles.
  B:  per query chain (hi0..hi3, lo3..lo0): interleaved accumulation
      out0/out1/l over the chain's pt tiles, then out = av * (1/l).
"""
import sys

sys.path.insert(0, "/opt/trn_rl_repo")

import ml_dtypes
import numpy as np

import concourse.bass as bass  # noqa: F401
import concourse.mybir as mybir
import concourse.tile as tile
from concourse import bacc
from concourse.bass_utils import run_bass_kernel_spmd

B, S, D = 4, 2048, 1024
DC = D // 128          # 8 contraction chunks
F32 = mybir.dt.float32
BF = mybir.dt.bfloat16
E4 = mybir.dt.float8e4
E4NP = ml_dtypes.float8_e4m3
BFNP = ml_dtypes.bfloat16
DR = mybir.MatmulPerfMode.DoubleRow
EXP = mybir.ActivationFunctionType.Exp
GROUPS = [[0, 1], [2, 3], [4, 5], [6, 7]]
EXP_SCALE = 1.0 / 32768.0   # q,k both carry x32; scores carry x1024*32

_cache = {}


def _tiles_at_kb(kb):
    """Phase A tiles that consume key block kb: list of (kind, i)."""
    out = []
    for i in range(4):
        if kb < 2 * i + 2:
            out.append(("sh", i))
        elif kb < 16 - 2 * i:
            out.append(("so", i))
    return out


def build_nc():
    nc = bacc.Bacc("TRN2", target_bir_lowering=False, debug=False, num_devices=8)
    # inputs, partition-major & contiguous per planned DMA
    xT = nc.dram_tensor("xT", [128, 4, 2, 4, 512], BF, kind="ExternalInput")
    xTq = nc.dram_tensor("xTq", [128, DC, 1024], BF, kind="ExternalInput")
    WqT = nc.dram_tensor("WqT", [128, DC, D], BF, kind="ExternalInput")
    WkTh = nc.dram_tensor("WkTh", [128, 4, DC, 128], BF, kind="ExternalInput")
    WvTh = nc.dram_tensor("WvTh", [128, 2, 4, 512], BF, kind="ExternalInput")
    masks_sh = nc.dram_tensor("masks_sh", [128, 8, 256], BF, kind="ExternalInput")
    masks_so = nc.dram_tensor("masks_so", [128, 8, 128], BF, kind="ExternalInput")
    out = nc.dram_tensor("out", [1024, D], F32, kind="ExternalOutput")
    # collective buffers
    kg_in = nc.dram_tensor("kg_in", [128, 4, S], E4)
    kg_out = nc.dram_tensor("kg_out", [2, 128, 4, S], E4)
    vg_in = nc.dram_tensor("vg_in", [128, 16, 512], BF)
    vg_out = nc.dram_tensor("vg_out", [2, 128, 16, 512], BF)

    with tile.TileContext(nc) as tc:
        with (
            tc.tile_pool(name="w", bufs=1) as wp,
            tc.tile_pool(name="xs", bufs=1) as xsp,
            tc.tile_pool(name="per", bufs=1) as per,
            tc.tile_pool(name="pt", bufs=1) as ptp,
            tc.tile_pool(name="ot", bufs=2) as otp,
            tc.tile_pool(name="sml", bufs=4) as smlp,
            tc.tile_pool(name="mix", bufs=4, space="PSUM") as mixp,
            tc.tile_pool(name="psav", bufs=4, space="PSUM") as psavp,
        ):
            # ---------------- consts + persistent ----------------
            zeros_f = per.tile([128, 2], F32)
            ones = per.tile([128, 2], BF)
            nc.vector.memset(zeros_f, 0.0)
            # exp(0)=1 -> also preloads the ACT exp table long before A
            nc.scalar.activation(ones, zeros_f, EXP)

            # x split by (sc, dc-half); wv by dc-half: fine-grained head DMAs
            wv = [wp.tile([128, 4, 512], BF, name=f"wv_{h}") for h in range(2)]
            xs = [
                [xsp.tile([128, 4, 512], BF, name=f"xs_{sc}_{h}") for h in range(2)]
                for sc in range(4)
            ]
            nc.sync.dma_start(out=wv[0], in_=WvTh[:, 0])
            nc.sync.dma_start(out=xs[0][0], in_=xT[:, 0, 0])
            nc.sync.dma_start(out=wv[1], in_=WvTh[:, 1])
            nc.sync.dma_start(out=xs[0][1], in_=xT[:, 0, 1])
            for sc in range(1, 4):
                for h in range(2):
                    nc.sync.dma_start(out=xs[sc][h], in_=xT[:, sc, h])
            wk = [wp.tile([128, DC, 128], BF, name=f"wk_{o}") for o in range(4)]
            for o in range(4):
                nc.sync.dma_start(out=wk[o], in_=WkTh[:, o])
            wq = wp.tile([128, DC, D], BF)
            xq = wp.tile([128, DC, 1024], BF)
            nc.sync.dma_start(out=wq, in_=WqT[:])
            nc.sync.dma_start(out=xq, in_=xTq[:])
            maskt_sh = per.tile([128, 8, 256], BF)
            maskt_so = per.tile([128, 8, 128], BF)
            nc.sync.dma_start(out=maskt_sh, in_=masks_sh[:])
            nc.sync.dma_start(out=maskt_so, in_=masks_so[:])

            # -------- P2: V half-projection (bf16), gather early --------
            vg_sb = per.tile([128, 16, 512], BF)
            for sc in range(4):
                for sb in range(4):
                    ps = mixp.tile([128, 512], F32, tag="mix", name=f"ps2_{sc}_{sb}")
                    for dc in range(DC):
                        nc.tensor.matmul(
                            ps,
                            lhsT=xs[sc][dc // 4][:, dc % 4, sb * 128:(sb + 1) * 128],
                            rhs=wv[dc // 4][:, dc % 4, :],
                            start=(dc == 0),
                            stop=(dc == DC - 1),
                        )
                    nc.vector.tensor_copy(vg_sb[:, sc * 4 + sb, :], ps)
            nc.scalar.dma_start(out=vg_in[:, 0:8, :], in_=vg_sb[:, 0:8, :])
            nc.scalar.dma_start(out=vg_in[:, 8:16, :], in_=vg_sb[:, 8:16, :])
            nc.gpsimd.collective_compute(
                "AllGather",
                mybir.AluOpType.bypass,
                replica_groups=GROUPS,
                ins=[vg_in[:]],
                outs=[vg_out[:]],
            )

            # -------- P1: K half-projection (fp8 out, x32 folded) --------
            kg_sb = per.tile([128, 4, S], E4)
            for sc in range(4):
                for ocl in range(4):
                    ps = mixp.tile([128, 512], F32, tag="mix")
                    for dc in range(DC):
                        nc.tensor.matmul(
                            ps,
                            lhsT=wk[ocl][:, dc, :],
                            rhs=xs[sc][dc // 4][:, dc % 4, :],
                            start=(dc == 0),
                            stop=(dc == DC - 1),
                        )
                    nc.vector.tensor_copy(
                        kg_sb[:, ocl, sc * 512:(sc + 1) * 512], ps
                    )
            nc.scalar.dma_start(out=kg_in[:, 0:2, :], in_=kg_sb[:, 0:2, :])
            nc.scalar.dma_start(out=kg_in[:, 2:4, :], in_=kg_sb[:, 2:4, :])
            nc.gpsimd.collective_compute(
                "AllGather",
                mybir.AluOpType.bypass,
                replica_groups=GROUPS,
                ins=[kg_in[:]],
                outs=[kg_out[:]],
            )

            # -------- load gathered vv (early: Phase B needs it) --------
            vv = per.tile([128, 2, 16, 512], BF)
            for r in range(2):
                nc.sync.dma_start(out=vv[:, r, 0:8], in_=vg_out[r][:, 0:8, :])
                nc.sync.dma_start(out=vv[:, r, 8:16], in_=vg_out[r][:, 8:16, :])

            # -------- P0: Q projection -> qt8 (fp8, overlaps gathers) ----
            qt8 = per.tile([128, DC, 1024], E4)
            for oc in range(8):
                pss = [
                    mixp.tile([128, 512], F32, tag="mix", name=f"ps0_{oc}_{i}")
                    for i in range(2)
                ]
                for dc in range(DC):
                    for sc in range(2):
                        nc.tensor.matmul(
                            pss[sc],
                            lhsT=wq[:, dc, oc * 128:(oc + 1) * 128],
                            rhs=xq[:, dc, sc * 512:(sc + 1) * 512],
                            start=(dc == 0),
                            stop=(dc == DC - 1),
                        )
                for sc in range(2):
                    nc.vector.tensor_copy(
                        qt8[:, oc, sc * 512:(sc + 1) * 512], pss[sc]
                    )

            # -------- load gathered kt8 --------
            kt8 = per.tile([128, DC, S], E4)
            for r in range(2):
                for ch in range(2):
                    nc.sync.dma_start(
                        out=kt8[:, 4 * r:4 * r + 4, ch * 1024:(ch + 1) * 1024],
                        in_=kg_out[r][:, :, ch * 1024:(ch + 1) * 1024],
                    )

            # ------- Phase A: scoresT + exp + mask, kb-major -------
            # per (kb, j) the kt8 weight slice is shared by every pair i
            # whose key range covers kb (<=4 interleaved PSUM groups).
            pts = {}
            for kb in range(16):
                users = _tiles_at_kb(kb)
                pss = {}
                for kind, i in users:
                    pss[(kind, i)] = mixp.tile(
                        [128, 512], F32, tag="mix", name=f"psA_{kb}_{kind}_{i}"
                    )
                for j in range(4):
                    for kind, i in users:
                        qc = 256 * i
                        if kind == "sh":
                            o, w = ps_slice = (0, 256)
                            rhs = qt8[:, 2 * j:2 * j + 2, qc:qc + 256]
                        else:
                            o, w = ps_slice = (0, 128)
                            rhs = qt8[:, 2 * j:2 * j + 2, qc + 128:qc + 256]
                        nc.tensor.matmul(
                            pss[(kind, i)][:, o:o + w],
                            lhsT=kt8[:, 2 * j:2 * j + 2, kb * 128:(kb + 1) * 128],
                            rhs=rhs,
                            start=(j == 0),
                            stop=(j == 3),
                            perf_mode=DR,
                        )
                for kind, i in users:
                    ps = pss[(kind, i)]
                    if kind == "sh":
                        pt = ptp.tile([128, 256], BF, name=f"ptsh_{i}_{kb}")
                        nc.scalar.activation(pt, ps[:, 0:256], EXP, scale=EXP_SCALE)
                        if kb >= 2 * i:
                            nc.vector.tensor_mul(pt, pt, maskt_sh[:, kb, :])
                    else:
                        pt = ptp.tile([128, 128], BF, name=f"ptso_{i}_{kb}")
                        nc.scalar.activation(pt, ps[:, 0:128], EXP, scale=EXP_SCALE)
                        if kb >= 14 - 2 * i:
                            nc.vector.tensor_mul(
                                pt, pt, maskt_so[:, kb - 14 + 4 * i, :]
                            )
                    pts[(kind, i, kb)] = pt

            # ---------------- Phase B: chains ----------------
            chains = []
            for i in range(4):  # hi chains, longest first
                tiles = [(("sh", i, kb), slice(128, 256), kb)
                         for kb in range(0, 2 * i + 2)]
                tiles += [(("so", i, kb), slice(0, 128), kb)
                          for kb in range(2 * i + 2, 16 - 2 * i)]
                chains.append(tiles)
            for i in (3, 2, 1, 0):  # lo chains, shortest last
                chains.append([(("sh", i, kb), slice(0, 128), kb)
                               for kb in range(0, 2 * i + 2)])

            for ci, tiles in enumerate(chains):
                avs = [
                    psavp.tile([128, 512], F32, tag="psav", name=f"av_{ci}_{oh}")
                    for oh in range(2)
                ]
                lps = psavp.tile([128, 2], F32, tag="psav", name=f"l_{ci}")
                n = len(tiles)
                for idx, (key, qsl, kb) in enumerate(tiles):
                    pt = pts[key]
                    first, last = idx == 0, idx == n - 1
                    for oh in range(2):
                        nc.tensor.matmul(
                            avs[oh],
                            lhsT=pt[:, qsl],
                            rhs=vv[:, oh, kb, :],
                            start=first,
                            stop=last,
                        )
                    nc.tensor.matmul(
                        lps, lhsT=pt[:, qsl], rhs=ones, start=first, stop=last
                    )
                rec = smlp.tile([128, 1], F32, tag="rec")
                nc.vector.reciprocal(rec, lps[:, 0:1])
                ot = otp.tile([128, D], F32, tag="ot")
                for oh in range(2):
                    nc.vector.tensor_scalar_mul(
                        ot[:, oh * 512:(oh + 1) * 512], avs[oh], rec
                    )
                    eng = nc.scalar if oh == 0 else nc.sync
                    eng.dma_start(
                        out=out[ci * 128:(ci + 1) * 128,
                                oh * 512:(oh + 1) * 512],
                        in_=ot[:, oh * 512:(oh + 1) * 512],
                    )
    nc.compile()
    return nc


def _pair_blocks(sub):
    """(lo_i, hi_i) query-block ids for pairs i=0..3."""
    return [(2 * i + sub, 15 - 2 * i - sub) for i in range(4)]


def _query_cols(sub):
    """qt/xTq column order: [lo0, hi0, lo1, hi1, ...] x 128 each."""
    cols = []
    for lo, hi in _pair_blocks(sub):
        cols.append(np.arange(lo * 128, lo * 128 + 128))
        cols.append(np.arange(hi * 128, hi * 128 + 128))
    return np.concatenate(cols)


def _chain_blocks(sub):
    """Output row order: chains hi0..hi3 then lo3..lo0."""
    pb = _pair_blocks(sub)
    return [pb[i][1] for i in range(4)] + [pb[i][0] for i in (3, 2, 1, 0)]


def _masks(sub):
    """masks_sh [128, 8, 256] and masks_so [128, 8, 128] (slot = 2i+d)."""
    p = np.arange(128)[:, None]
    j = np.arange(128)[None, :]
    msh = np.zeros((8, 128, 256), np.float32)
    mso = np.zeros((8, 128, 128), np.float32)
    for i, (lo, hi) in enumerate(_pair_blocks(sub)):
        for d in range(2):
            kb = 2 * i + d
            msh[2 * i + d, :, 0:128] = (kb * 128 + p <= lo * 128 + j)
            msh[2 * i + d, :, 128:256] = (kb * 128 + p <= hi * 128 + j)
            kbs = 14 - 2 * i + d
            mso[2 * i + d] = (kbs * 128 + p <= hi * 128 + j)
    return (
        np.ascontiguousarray(msh.transpose(1, 0, 2)),
        np.ascontiguousarray(mso.transpose(1, 0, 2)),
    )


def _pmaj(a):
    """[dc*128, cols] -> partition-major [128, dc, cols]."""
    d, cols = a.shape
    return np.ascontiguousarray(a.reshape(d // 128, 128, cols).transpose(1, 0, 2))


def kernel(x, Wq, Wk, Wv, _trace=False):
    if "nc" not in _cache:
        _cache["nc"] = build_nc()
    nc = _cache["nc"]

    x = np.asarray(x, dtype=np.float32)
    # q' = 32q, k' = 32k: x32 into Wq (net of the folded 1/sqrt(d)) and Wk;
    # scores then carry 32*32*32 = 2^15, removed by EXP_SCALE.
    WqTs = _pmaj((np.asarray(Wq, np.float32).T * np.float32(32.0)).astype(BFNP))
    WkTs = (np.asarray(Wk, np.float32).T * np.float32(32.0)).astype(BFNP)
    WvT = np.asarray(Wv, np.float32).T.astype(BFNP)

    in_maps = []
    for c in range(8):
        b, sub = c // 2, c % 2
        xTb = x[b].T.astype(BFNP)                      # [1024, 2048]
        xTp = _pmaj(xTb)                               # [128, 8, 2048]
        xT4 = np.ascontiguousarray(
            xTp.reshape(128, 2, 4, 4, 512).transpose(0, 3, 1, 2, 4)
        )                                              # [128, sc, h, dc%4, 512]
        wkh = _pmaj(WkTs[:, sub * 512:(sub + 1) * 512])  # [128, 8, 512]
        wk4 = np.ascontiguousarray(
            wkh.reshape(128, DC, 4, 128).transpose(0, 2, 1, 3)
        )                                              # [128, 4, 8, 128]
        wvh = _pmaj(WvT[:, sub * 512:(sub + 1) * 512])   # [128, 8, 512]
        wv2 = np.ascontiguousarray(wvh.reshape(128, 2, 4, 512))
        msh, mso = _masks(sub)
        in_maps.append(
            {
                "xT": xT4,
                "xTq": _pmaj(np.ascontiguousarray(xTb[:, _query_cols(sub)])),
                "WqT": WqTs,
                "WkTh": wk4,
                "WvTh": wv2,
                "masks_sh": msh.astype(BFNP),
                "masks_so": mso.astype(BFNP),
            }
        )

    res = run_bass_kernel_spmd(
        nc, in_maps, core_ids=list(range(8)), trace=_trace
    )
    full = np.empty((B, S, D), np.float32)
    for c in range(8):
        b, sub = c // 2, c % 2
        for pos, qb in enumerate(_chain_blocks(sub)):
            full[b, qb * 128:(qb + 1) * 128] = (
                res.results[c]["out"][pos * 128:(pos + 1) * 128]
            )
    if _trace:
        _cache["last_result"] = res
    return full


# revision 5
# speedup vs baseline: 1.1775x; 1.1775x over previous
"""Causal single-head attention (B=4, S=2048, d=1024) on 8 TRN2 NeuronCores.

Sharding: core c -> batch b = c//2, subset s = c%2. Per batch the 16
query blocks (128 rows) are assigned in balanced causal pairs: core
(b,s) owns pairs (lo_i, hi_i) = (2i+s, 15-2i-s), i=0..3, giving every
core 68 true causal score tiles (padded to a uniform 72). Every core
runs the identical instruction stream; causal boundaries come from
per-core 0/1 mask tiles supplied as input data.

K/V projections are tensor-parallel within each core pair: core (b,s)
computes the d_out-half s of kT (fp8, x32-scaled) and v (bf16) for the
whole batch; halves are exchanged with pairwise AllGathers
([[0,1],[2,3],[4,5],[6,7]]). The gathers are ~25us latency-bound and
serialize on the CC core, so K's gather goes first (Phase A needs it
~20us before Phase B needs V's).

Precision: projections and AV run in bf16 (fp32 PSUM). Scores run in
fp8e4m3 DoubleRow (256-deep contraction per pass = 2x bf16 FLOPs):
host folds x32 into Wq and Wk so q' = 32q, k' = 32k sit in the fp8
sweet spot; the combined 2^15 scale is removed inside the EXP
activation (exp(s' / 32768)).

qt8 column layout is [lo0 lo1 lo2 lo3 hi0 hi1 hi2 hi3] (128 cols per
block). At key block kb the live lo blocks are the suffix i >= kb//2
(kb < 8) and the live hi blocks are the prefix i < h(kb); both are
contiguous column ranges, so Phase A needs only TWO wide DR matmuls
per (kb, dc-pair) into two PSUM tiles. Phase B chains slice columns
out of the per-kb exp tiles.
"""
import sys

sys.path.insert(0, "/opt/trn_rl_repo")

import ml_dtypes
import numpy as np

import concourse.bass as bass  # noqa: F401
import concourse.mybir as mybir
import concourse.tile as tile
from concourse import bacc
from concourse.bass_utils import run_bass_kernel_spmd

B, S, D = 4, 2048, 1024
DC = D // 128          # 8 contraction chunks
F32 = mybir.dt.float32
BF = mybir.dt.bfloat16
E4 = mybir.dt.float8e4
E4NP = ml_dtypes.float8_e4m3
BFNP = ml_dtypes.bfloat16
DR = mybir.MatmulPerfMode.DoubleRow
EXP = mybir.ActivationFunctionType.Exp
COPY = mybir.ActivationFunctionType.Copy
GROUPS = [[0, 1], [2, 3], [4, 5], [6, 7]]
EXP_SCALE = 1.0 / 32768.0   # q,k both carry x32; scores carry x1024*32


def _i0(kb):
    """First live lo block at key block kb (suffix i0..3), kb < 8."""
    return kb // 2


def _h(kb):
    """Number of live hi blocks at key block kb (prefix 0..h-1)."""
    return min(4, (17 - kb) // 2)


_cache = {}


def build_nc():
    nc = bacc.Bacc("TRN2", target_bir_lowering=False, debug=False, num_devices=8)
    # inputs, partition-major & contiguous per planned DMA
    xT = nc.dram_tensor("xT", [128, 4, 2, 4, 512], BF, kind="ExternalInput")
    xTq = nc.dram_tensor("xTq", [128, DC, 1024], BF, kind="ExternalInput")
    WqT = nc.dram_tensor("WqT", [128, DC, D], BF, kind="ExternalInput")
    WkTh = nc.dram_tensor("WkTh", [128, 4, DC, 128], BF, kind="ExternalInput")
    WvTh = nc.dram_tensor("WvTh", [128, 2, 4, 512], BF, kind="ExternalInput")
    masks_lo = nc.dram_tensor("masks_lo", [128, 8, 512], BF, kind="ExternalInput")
    masks_hi = nc.dram_tensor("masks_hi", [128, 8, 512], BF, kind="ExternalInput")
    out = nc.dram_tensor("out", [1024, D], F32, kind="ExternalOutput")
    # collective buffers
    kg_in = nc.dram_tensor("kg_in", [128, 4, S], E4)
    kg_out = nc.dram_tensor("kg_out", [2, 128, 4, S], E4)
    vg_in = nc.dram_tensor("vg_in", [128, 16, 512], BF)
    vg_out = nc.dram_tensor("vg_out", [2, 128, 16, 512], BF)

    with tile.TileContext(nc) as tc:
        with (
            tc.tile_pool(name="w", bufs=1) as wp,
            tc.tile_pool(name="xs", bufs=1) as xsp,
            tc.tile_pool(name="per", bufs=1) as per,
            tc.tile_pool(name="pt", bufs=1) as ptp,
            tc.tile_pool(name="ot", bufs=2) as otp,
            tc.tile_pool(name="sml", bufs=4) as smlp,
            tc.tile_pool(name="mix", bufs=4, space="PSUM") as mixp,
            tc.tile_pool(name="psav", bufs=4, space="PSUM") as psavp,
        ):
            # ---------------- consts + persistent ----------------
            zeros_f = per.tile([128, 2], F32)
            ones = per.tile([128, 2], BF)
            nc.vector.memset(zeros_f, 0.0)
            # exp(0)=1 -> also preloads the ACT exp table long before A
            nc.scalar.activation(ones, zeros_f, EXP)

            # fine-grained head DMAs: first K-proj group needs wk_0 + xs_0_0
            wk = [wp.tile([128, DC, 128], BF, name=f"wk_{o}") for o in range(4)]
            xs = [
                [xsp.tile([128, 4, 512], BF, name=f"xs_{sc}_{h}") for h in range(2)]
                for sc in range(4)
            ]
            nc.sync.dma_start(out=wk[0], in_=WkTh[:, 0])
            nc.sync.dma_start(out=xs[0][0], in_=xT[:, 0, 0])
            for o in range(1, 4):
                nc.sync.dma_start(out=wk[o], in_=WkTh[:, o])
            nc.sync.dma_start(out=xs[0][1], in_=xT[:, 0, 1])
            for sc in range(1, 4):
                for h in range(2):
                    nc.sync.dma_start(out=xs[sc][h], in_=xT[:, sc, h])
            wv = [wp.tile([128, 4, 512], BF, name=f"wv_{h}") for h in range(2)]
            for h in range(2):
                nc.sync.dma_start(out=wv[h], in_=WvTh[:, h])
            wq = wp.tile([128, DC, D], BF)
            xq = wp.tile([128, DC, 1024], BF)
            nc.sync.dma_start(out=wq, in_=WqT[:])
            nc.sync.dma_start(out=xq, in_=xTq[:])
            maskt_lo = per.tile([128, 8, 512], BF)
            maskt_hi = per.tile([128, 8, 512], BF)
            nc.sync.dma_start(out=maskt_lo, in_=masks_lo[:])
            nc.sync.dma_start(out=maskt_hi, in_=masks_hi[:])

            # -------- P1: K half-projection (fp8 out, x32 folded) --------
            kg_sb = per.tile([128, 4, S], E4)
            for sc in range(4):
                for ocl in range(4):
                    ps = mixp.tile([128, 512], F32, tag="mix")
                    for dc in range(DC):
                        nc.tensor.matmul(
                            ps,
                            lhsT=wk[ocl][:, dc, :],
                            rhs=xs[sc][dc // 4][:, dc % 4, :],
                            start=(dc == 0),
                            stop=(dc == DC - 1),
                        )
                    nc.vector.tensor_copy(
                        kg_sb[:, ocl, sc * 512:(sc + 1) * 512], ps
                    )
            nc.scalar.dma_start(out=kg_in[:, 0:2, :], in_=kg_sb[:, 0:2, :])
            nc.scalar.dma_start(out=kg_in[:, 2:4, :], in_=kg_sb[:, 2:4, :])
            nc.gpsimd.collective_compute(
                "AllGather",
                mybir.AluOpType.bypass,
                replica_groups=GROUPS,
                ins=[kg_in[:]],
                outs=[kg_out[:]],
            )

            # -------- P2: V half-projection (bf16) --------
            vg_sb = per.tile([128, 16, 512], BF)
            for sc in range(4):
                for sb in range(4):
                    ps = mixp.tile([128, 512], F32, tag="mix", name=f"ps2_{sc}_{sb}")
                    for dc in range(DC):
                        nc.tensor.matmul(
                            ps,
                            lhsT=xs[sc][dc // 4][:, dc % 4, sb * 128:(sb + 1) * 128],
                            rhs=wv[dc // 4][:, dc % 4, :],
                            start=(dc == 0),
                            stop=(dc == DC - 1),
                        )
                    nc.vector.tensor_copy(vg_sb[:, sc * 4 + sb, :], ps)
            nc.scalar.dma_start(out=vg_in[:, 0:8, :], in_=vg_sb[:, 0:8, :])
            nc.scalar.dma_start(out=vg_in[:, 8:16, :], in_=vg_sb[:, 8:16, :])
            nc.gpsimd.collective_compute(
                "AllGather",
                mybir.AluOpType.bypass,
                replica_groups=GROUPS,
                ins=[vg_in[:]],
                outs=[vg_out[:]],
            )

            # -------- load gathered kt8 (key-halves so A starts early) ---
            kt8 = [wp.tile([128, DC, 1024], E4, name=f"kt8_{ch}") for ch in range(2)]
            for ch in range(2):
                for r in range(2):
                    nc.sync.dma_start(
                        out=kt8[ch][:, 4 * r:4 * r + 4, :],
                        in_=kg_out[r][:, :, ch * 1024:(ch + 1) * 1024],
                    )

            # -------- P0: Q projection -> qt8 (fp8, overlaps gathers) ----
            qt8 = per.tile([128, DC, 1024], E4)
            for oc in range(8):
                pss = [
                    mixp.tile([128, 512], F32, tag="mix", name=f"ps0_{oc}_{i}")
                    for i in range(2)
                ]
                for dc in range(DC):
                    for sc in range(2):
                        nc.tensor.matmul(
                            pss[sc],
                            lhsT=wq[:, dc, oc * 128:(oc + 1) * 128],
                            rhs=xq[:, dc, sc * 512:(sc + 1) * 512],
                            start=(dc == 0),
                            stop=(dc == DC - 1),
                        )
                for sc in range(2):
                    nc.vector.tensor_copy(
                        qt8[:, oc, sc * 512:(sc + 1) * 512], pss[sc]
                    )

            # -------- load gathered vv (key-halves; lo chains only need a)
            vva = per.tile([128, 2, 8, 512], BF)
            vvb = per.tile([128, 2, 8, 512], BF)
            for r in range(2):
                nc.sync.dma_start(out=vva[:, r], in_=vg_out[r][:, 0:8, :])
            for r in range(2):
                nc.sync.dma_start(out=vvb[:, r], in_=vg_out[r][:, 8:16, :])

            def vv_rhs(oh, kb):
                t = vva if kb < 8 else vvb
                return t[:, oh, kb % 8, :]

            # ------- Phase A: scoresT + exp + mask, two wide tiles per kb
            pt_lo = {}
            pt_hi = {}
            for kb in range(16):
                ktc = kt8[kb // 8]
                kcol = (kb % 8) * 128
                h = _h(kb)
                tiles = [("hi", 512, 128 * h)]
                if kb < 8:
                    i0 = _i0(kb)
                    tiles.append(("lo", 128 * i0, 128 * (4 - i0)))
                pss = {}
                for kind, qo, qw in tiles:
                    pss[kind] = mixp.tile(
                        [128, 512], F32, tag="mix", name=f"psA_{kb}_{kind}"
                    )
                for j in range(4):
                    for kind, qo, qw in tiles:
                        nc.tensor.matmul(
                            pss[kind][:, 0:qw],
                            lhsT=ktc[:, 2 * j:2 * j + 2, kcol:kcol + 128],
                            rhs=qt8[:, 2 * j:2 * j + 2, qo:qo + qw],
                            start=(j == 0),
                            stop=(j == 3),
                            perf_mode=DR,
                        )
                for kind, qo, qw in tiles:
                    pt = ptp.tile([128, qw], BF, name=f"pt_{kind}_{kb}")
                    nc.scalar.activation(pt, pss[kind][:, 0:qw], EXP,
                                         scale=EXP_SCALE)
                    if kind == "lo":
                        nc.vector.tensor_mul(pt, pt, maskt_lo[:, kb, 0:qw])
                        pt_lo[kb] = pt
                    else:
                        if kb >= 8:
                            nc.vector.tensor_mul(
                                pt, pt, maskt_hi[:, kb - 8, 0:qw]
                            )
                        pt_hi[kb] = pt

            # ---------------- Phase B: chains ----------------
            # lo chains first (need only vva), then hi chains.
            chains = []
            for i in (3, 2, 1, 0):
                chains.append([
                    (pt_lo[kb], 128 * (i - _i0(kb)), kb)
                    for kb in range(0, 2 * i + 2)
                ])
            for i in range(4):
                chains.append([
                    (pt_hi[kb], 128 * i, kb) for kb in range(0, 16 - 2 * i)
                ])

            for ci, tiles in enumerate(chains):
                avs = [
                    psavp.tile([128, 512], F32, tag="psav", name=f"av_{ci}_{oh}")
                    for oh in range(2)
                ]
                lps = psavp.tile([128, 2], F32, tag="psav", name=f"l_{ci}")
                n = len(tiles)
                for idx, (pt, qo, kb) in enumerate(tiles):
                    first, last = idx == 0, idx == n - 1
                    for oh in range(2):
                        nc.tensor.matmul(
                            avs[oh],
                            lhsT=pt[:, qo:qo + 128],
                            rhs=vv_rhs(oh, kb),
                            start=first,
                            stop=last,
                        )
                    nc.tensor.matmul(
                        lps, lhsT=pt[:, qo:qo + 128], rhs=ones,
                        start=first, stop=last,
                    )
                rec = smlp.tile([128, 1], F32, tag="rec")
                nc.vector.reciprocal(rec, lps[:, 0:1])
                ot = otp.tile([128, D], F32, tag="ot")
                nc.vector.tensor_scalar_mul(ot[:, 0:512], avs[0], rec)
                nc.scalar.activation(ot[:, 512:1024], avs[1], COPY, scale=rec)
                nc.scalar.dma_start(
                    out=out[ci * 128:(ci + 1) * 128, 0:512], in_=ot[:, 0:512]
                )
                nc.sync.dma_start(
                    out=out[ci * 128:(ci + 1) * 128, 512:1024],
                    in_=ot[:, 512:1024],
                )
    nc.compile()
    return nc


def _pair_blocks(sub):
    """(lo_i, hi_i) query-block ids for pairs i=0..3."""
    return [(2 * i + sub, 15 - 2 * i - sub) for i in range(4)]


def _query_cols(sub):
    """qt/xTq column order: [lo0 lo1 lo2 lo3 hi0 hi1 hi2 hi3] x 128."""
    pb = _pair_blocks(sub)
    cols = [np.arange(lo * 128, lo * 128 + 128) for lo, _ in pb]
    cols += [np.arange(hi * 128, hi * 128 + 128) for _, hi in pb]
    return np.concatenate(cols)


def _chain_blocks(sub):
    """Output row order: chains lo3..lo0 then hi0..hi3."""
    pb = _pair_blocks(sub)
    return [pb[i][0] for i in (3, 2, 1, 0)] + [pb[i][1] for i in range(4)]


def _masks(sub):
    """masks_lo / masks_hi [128, 8, 512]: slot kb (lo) / kb-8 (hi)."""
    p = np.arange(128)[:, None]
    j = np.arange(128)[None, :]
    pb = _pair_blocks(sub)
    mlo = np.ones((8, 128, 512), np.float32)
    mhi = np.ones((8, 128, 512), np.float32)
    for kb in range(8):
        i0 = _i0(kb)
        for c, i in enumerate(range(i0, 4)):
            qb = pb[i][0]
            mlo[kb, :, c * 128:(c + 1) * 128] = (kb * 128 + p <= qb * 128 + j)
    for kb in range(8, 16):
        for i in range(_h(kb)):
            qb = pb[i][1]
            mhi[kb - 8, :, i * 128:(i + 1) * 128] = (
                kb * 128 + p <= qb * 128 + j
            )
    return (
        np.ascontiguousarray(mlo.transpose(1, 0, 2)),
        np.ascontiguousarray(mhi.transpose(1, 0, 2)),
    )


def _pmaj(a):
    """[dc*128, cols] -> partition-major [128, dc, cols]."""
    d, cols = a.shape
    return np.ascontiguousarray(a.reshape(d // 128, 128, cols).transpose(1, 0, 2))


def kernel(x, Wq, Wk, Wv, _trace=False):
    if "nc" not in _cache:
        _cache["nc"] = build_nc()
    nc = _cache["nc"]

    x = np.asarray(x, dtype=np.float32)
    # q' = 32q, k' = 32k: x32 into Wq (net of the folded 1/sqrt(d)) and Wk;
    # scores then carry 32*32*32 = 2^15, removed by EXP_SCALE.
    WqTs = _pmaj((np.asarray(Wq, np.float32).T * np.float32(32.0)).astype(BFNP))
    WkTs = (np.asarray(Wk, np.float32).T * np.float32(32.0)).astype(BFNP)
    WvT = np.asarray(Wv, np.float32).T.astype(BFNP)

    in_maps = []
    for c in range(8):
        b, sub = c // 2, c % 2
        xTb = x[b].T.astype(BFNP)                      # [1024, 2048]
        xTp = _pmaj(xTb)                               # [128, 8, 2048]
        xT4 = np.ascontiguousarray(
            xTp.reshape(128, 2, 4, 4, 512).transpose(0, 3, 1, 2, 4)
        )                                              # [128, sc, h, dc%4, 512]
        wkh = _pmaj(WkTs[:, sub * 512:(sub + 1) * 512])  # [128, 8, 512]
        wk4 = np.ascontiguousarray(
            wkh.reshape(128, DC, 4, 128).transpose(0, 2, 1, 3)
        )                                              # [128, 4, 8, 128]
        wvh = _pmaj(WvT[:, sub * 512:(sub + 1) * 512])   # [128, 8, 512]
        wv2 = np.ascontiguousarray(wvh.reshape(128, 2, 4, 512))
        mlo, mhi = _masks(sub)
        in_maps.append(
            {
                "xT": xT4,
                "xTq": _pmaj(np.ascontiguousarray(xTb[:, _query_cols(sub)])),
                "WqT": WqTs,
                "WkTh": wk4,
                "WvTh": wv2,
                "masks_lo": mlo.astype(BFNP),
                "masks_hi": mhi.astype(BFNP),
            }
        )

    res = run_bass_kernel_spmd(
        nc, in_maps, core_ids=list(range(8)), trace=_trace
    )
    full = np.empty((B, S, D), np.float32)
    for c in range(8):
        b, sub = c // 2, c % 2
        for pos, qb in enumerate(_chain_blocks(sub)):
            full[b, qb * 128:(qb + 1) * 128] = (
                res.results[c]["out"][pos * 128:(pos + 1) * 128]
            )
    if _trace:
        _cache["last_result"] = res
    return full


# revision 8
# speedup vs baseline: 1.2107x; 1.0282x over previous
"""Causal single-head attention (B=4, S=2048, d=1024) on 8 TRN2 NeuronCores.

Sharding: core c -> batch b = c//2, subset s = c%2. Per batch the 16
query blocks (128 rows) are assigned in balanced causal pairs: core
(b,s) owns pairs (lo_i, hi_i) = (2i+s, 15-2i-s), i=0..3, giving every
core 68 true causal score tiles (padded to a uniform 72). Every core
runs the identical instruction stream; causal boundaries come from
per-core 0/1 mask tiles supplied as input data.

K/V projections are tensor-parallel within each core pair: core (b,s)
computes the d_out-half s of kT (fp8, x32-scaled) and v (bf16) for the
whole batch; halves are exchanged with pairwise AllGathers
([[0,1],[2,3],[4,5],[6,7]]). The gathers are ~25us latency-bound and
serialize on the CC core, so K's gather goes first (Phase A needs it
~20us before Phase B needs V's).

Precision: projections and AV run in bf16 (fp32 PSUM). Scores run in
fp8e4m3 DoubleRow (256-deep contraction per pass = 2x bf16 FLOPs):
host folds x32 into Wq and Wk so q' = 32q, k' = 32k sit in the fp8
sweet spot; the combined 2^15 scale is removed inside the EXP
activation (exp(s' / 32768)).

qt8 column layout is [lo0 lo1 lo2 lo3 hi0 hi1 hi2 hi3] (128 cols per
block). At key block kb the live lo blocks are the suffix i >= kb//2
(kb < 8) and the live hi blocks are the prefix i < h(kb); both are
contiguous column ranges, so Phase A needs only TWO wide DR matmuls
per (kb, dc-pair) into two PSUM tiles. Phase B chains slice columns
out of the per-kb exp tiles.
"""
import sys

sys.path.insert(0, "/opt/trn_rl_repo")

import ml_dtypes
import numpy as np

import concourse.bass as bass  # noqa: F401
import concourse.mybir as mybir
import concourse.tile as tile
from concourse import bacc
from concourse.bass_utils import run_bass_kernel_spmd

B, S, D = 4, 2048, 1024
DC = D // 128          # 8 contraction chunks
F32 = mybir.dt.float32
BF = mybir.dt.bfloat16
E4 = mybir.dt.float8e4
E4NP = ml_dtypes.float8_e4m3
BFNP = ml_dtypes.bfloat16
DR = mybir.MatmulPerfMode.DoubleRow
EXP = mybir.ActivationFunctionType.Exp
COPY = mybir.ActivationFunctionType.Copy
GROUPS = [[0, 1], [2, 3], [4, 5], [6, 7]]
EXP_SCALE = 1.0 / 32768.0   # q,k both carry x32; scores carry x1024*32


def _i0(kb):
    """First live lo block at key block kb (suffix i0..3), kb < 8."""
    return kb // 2


def _h(kb):
    """Number of live hi blocks at key block kb (prefix 0..h-1)."""
    return min(4, (17 - kb) // 2)


_cache = {}


def build_nc():
    nc = bacc.Bacc("TRN2", target_bir_lowering=False, debug=False, num_devices=8)
    # inputs, partition-major & contiguous per planned DMA
    xT = nc.dram_tensor("xT", [128, 4, 2, 4, 512], BF, kind="ExternalInput")
    xTq = nc.dram_tensor("xTq", [128, DC, 1024], BF, kind="ExternalInput")
    WqT = nc.dram_tensor("WqT", [128, DC, D], BF, kind="ExternalInput")
    WkTh = nc.dram_tensor("WkTh", [128, 4, DC, 128], BF, kind="ExternalInput")
    WvTh = nc.dram_tensor("WvTh", [128, 2, 4, 512], BF, kind="ExternalInput")
    masks_lo = nc.dram_tensor("masks_lo", [128, 8, 512], BF, kind="ExternalInput")
    masks_hi = nc.dram_tensor("masks_hi", [128, 8, 512], BF, kind="ExternalInput")
    out = nc.dram_tensor("out", [1024, D], F32, kind="ExternalOutput")
    # collective buffers
    kg_in = nc.dram_tensor("kg_in", [128, 4, S], E4)
    kg_out = nc.dram_tensor("kg_out", [2, 128, 4, S], E4)
    vg_in = [
        nc.dram_tensor(f"vg_in{h}", [128, 8, 512], BF) for h in range(2)
    ]
    vg_out = [
        nc.dram_tensor(f"vg_out{h}", [2, 128, 8, 512], BF) for h in range(2)
    ]

    with tile.TileContext(nc) as tc:
        with (
            tc.tile_pool(name="w", bufs=1) as wp,
            tc.tile_pool(name="xs", bufs=1) as xsp,
            tc.tile_pool(name="per", bufs=1) as per,
            tc.tile_pool(name="pt", bufs=1) as ptp,
            tc.tile_pool(name="ot", bufs=2) as otp,
            tc.tile_pool(name="sml", bufs=4) as smlp,
            tc.tile_pool(name="mix", bufs=4, space="PSUM") as mixp,
            tc.tile_pool(name="psav", bufs=4, space="PSUM") as psavp,
        ):
            # ---------------- consts + persistent ----------------
            zeros_f = per.tile([128, 2], F32)
            ones = per.tile([128, 2], BF)
            nc.vector.memset(zeros_f, 0.0)
            # exp(0)=1 -> also preloads the ACT exp table long before A
            nc.scalar.activation(ones, zeros_f, EXP)

            # fine-grained head DMAs: first K-proj group needs wk_0 + xs_0_0
            wk = [wp.tile([128, DC, 128], BF, name=f"wk_{o}") for o in range(4)]
            xs = [
                [xsp.tile([128, 4, 512], BF, name=f"xs_{sc}_{h}") for h in range(2)]
                for sc in range(4)
            ]
            nc.sync.dma_start(out=wk[0], in_=WkTh[:, 0])
            nc.sync.dma_start(out=xs[0][0], in_=xT[:, 0, 0])
            for o in range(1, 4):
                nc.sync.dma_start(out=wk[o], in_=WkTh[:, o])
            nc.sync.dma_start(out=xs[0][1], in_=xT[:, 0, 1])
            for sc in range(1, 4):
                for h in range(2):
                    nc.sync.dma_start(out=xs[sc][h], in_=xT[:, sc, h])
            wv = [wp.tile([128, 4, 512], BF, name=f"wv_{h}") for h in range(2)]
            for h in range(2):
                nc.sync.dma_start(out=wv[h], in_=WvTh[:, h])
            wq = wp.tile([128, DC, D], BF)
            xq = wp.tile([128, DC, 1024], BF)
            nc.sync.dma_start(out=wq, in_=WqT[:])
            nc.sync.dma_start(out=xq, in_=xTq[:])
            maskt_lo = per.tile([128, 8, 512], BF)
            maskt_hi = per.tile([128, 8, 512], BF)
            nc.sync.dma_start(out=maskt_lo, in_=masks_lo[:])
            nc.sync.dma_start(out=maskt_hi, in_=masks_hi[:])

            # -------- P1: K half-projection (fp8 out, x32 folded) --------
            kg_sb = per.tile([128, 4, S], E4)
            for sc in range(4):
                for ocl in range(4):
                    ps = mixp.tile([128, 512], F32, tag="mix")
                    for dc in range(DC):
                        nc.tensor.matmul(
                            ps,
                            lhsT=wk[ocl][:, dc, :],
                            rhs=xs[sc][dc // 4][:, dc % 4, :],
                            start=(dc == 0),
                            stop=(dc == DC - 1),
                        )
                    nc.vector.tensor_copy(
                        kg_sb[:, ocl, sc * 512:(sc + 1) * 512], ps
                    )
            nc.scalar.dma_start(out=kg_in[:, 0:2, :], in_=kg_sb[:, 0:2, :])
            nc.scalar.dma_start(out=kg_in[:, 2:4, :], in_=kg_sb[:, 2:4, :])
            nc.gpsimd.collective_compute(
                "AllGather",
                mybir.AluOpType.bypass,
                replica_groups=GROUPS,
                ins=[kg_in[:]],
                outs=[kg_out[:]],
            )

            # -------- P2: V half-projection (bf16) --------
            # key-halves gathered separately: the first 1MB collective can
            # start at P2's midpoint, right after AG_K frees the CC core.
            vg_sb = [
                per.tile([128, 8, 512], BF, name=f"vg_sb{h}") for h in range(2)
            ]
            for sc in range(4):
                for sb in range(4):
                    ps = mixp.tile([128, 512], F32, tag="mix", name=f"ps2_{sc}_{sb}")
                    for dc in range(DC):
                        nc.tensor.matmul(
                            ps,
                            lhsT=xs[sc][dc // 4][:, dc % 4, sb * 128:(sb + 1) * 128],
                            rhs=wv[dc // 4][:, dc % 4, :],
                            start=(dc == 0),
                            stop=(dc == DC - 1),
                        )
                    nc.vector.tensor_copy(
                        vg_sb[sc // 2][:, (sc % 2) * 4 + sb, :], ps
                    )
                if sc % 2 == 1:
                    h = sc // 2
                    nc.scalar.dma_start(out=vg_in[h][:], in_=vg_sb[h])
                    nc.gpsimd.collective_compute(
                        "AllGather",
                        mybir.AluOpType.bypass,
                        replica_groups=GROUPS,
                        ins=[vg_in[h][:]],
                        outs=[vg_out[h][:]],
                    )

            # -------- load gathered kt8 (key-halves so A starts early) ---
            kt8 = [wp.tile([128, DC, 1024], E4, name=f"kt8_{ch}") for ch in range(2)]
            for ch in range(2):
                for r in range(2):
                    nc.sync.dma_start(
                        out=kt8[ch][:, 4 * r:4 * r + 4, :],
                        in_=kg_out[r][:, :, ch * 1024:(ch + 1) * 1024],
                    )

            # -------- P0: Q projection -> qt8 (fp8, overlaps gathers) ----
            qt8 = per.tile([128, DC, 1024], E4)
            for oc in range(8):
                pss = [
                    mixp.tile([128, 512], F32, tag="mix", name=f"ps0_{oc}_{i}")
                    for i in range(2)
                ]
                for dc in range(DC):
                    for sc in range(2):
                        nc.tensor.matmul(
                            pss[sc],
                            lhsT=wq[:, dc, oc * 128:(oc + 1) * 128],
                            rhs=xq[:, dc, sc * 512:(sc + 1) * 512],
                            start=(dc == 0),
                            stop=(dc == DC - 1),
                        )
                for sc in range(2):
                    nc.vector.tensor_copy(
                        qt8[:, oc, sc * 512:(sc + 1) * 512], pss[sc]
                    )

            # -------- load gathered vv (key-halves; lo chains only need a)
            vva = per.tile([128, 2, 8, 512], BF)
            vvb = per.tile([128, 2, 8, 512], BF)
            for r in range(2):
                nc.sync.dma_start(out=vva[:, r], in_=vg_out[0][r])
            for r in range(2):
                nc.sync.dma_start(out=vvb[:, r], in_=vg_out[1][r])

            def vv_rhs(oh, kb):
                t = vva if kb < 8 else vvb
                return t[:, oh, kb % 8, :]

            # ------- Phase A: scoresT + exp + mask, two wide tiles per kb
            pt_lo = {}
            pt_hi = {}
            for kb in range(16):
                ktc = kt8[kb // 8]
                kcol = (kb % 8) * 128
                h = _h(kb)
                tiles = [("hi", 512, 128 * h)]
                if kb < 8:
                    i0 = _i0(kb)
                    tiles.append(("lo", 128 * i0, 128 * (4 - i0)))
                pss = {}
                for kind, qo, qw in tiles:
                    pss[kind] = mixp.tile(
                        [128, 512], F32, tag="mix", name=f"psA_{kb}_{kind}"
                    )
                for j in range(4):
                    for kind, qo, qw in tiles:
                        nc.tensor.matmul(
                            pss[kind][:, 0:qw],
                            lhsT=ktc[:, 2 * j:2 * j + 2, kcol:kcol + 128],
                            rhs=qt8[:, 2 * j:2 * j + 2, qo:qo + qw],
                            start=(j == 0),
                            stop=(j == 3),
                            perf_mode=DR,
                        )
                for kind, qo, qw in tiles:
                    pt = ptp.tile([128, qw], BF, name=f"pt_{kind}_{kb}")
                    nc.scalar.activation(pt, pss[kind][:, 0:qw], EXP,
                                         scale=EXP_SCALE)
                    if kind == "lo":
                        nc.vector.tensor_mul(pt, pt, maskt_lo[:, kb, 0:qw])
                        pt_lo[kb] = pt
                    else:
                        if kb >= 8:
                            nc.vector.tensor_mul(
                                pt, pt, maskt_hi[:, kb - 8, 0:qw]
                            )
                        pt_hi[kb] = pt

            # ---------------- Phase B: chains ----------------
            # lo chains first (need only vva), then hi chains.
            chains = []
            for i in (3, 2, 1, 0):
                chains.append([
                    (pt_lo[kb], 128 * (i - _i0(kb)), kb)
                    for kb in range(0, 2 * i + 2)
                ])
            for i in range(4):
                chains.append([
                    (pt_hi[kb], 128 * i, kb) for kb in range(0, 16 - 2 * i)
                ])

            for ci, tiles in enumerate(chains):
                avs = [
                    psavp.tile([128, 512], F32, tag="psav", name=f"av_{ci}_{oh}")
                    for oh in range(2)
                ]
                lps = psavp.tile([128, 2], F32, tag="psav", name=f"l_{ci}")
                n = len(tiles)
                for idx, (pt, qo, kb) in enumerate(tiles):
                    first, last = idx == 0, idx == n - 1
                    for oh in range(2):
                        nc.tensor.matmul(
                            avs[oh],
                            lhsT=pt[:, qo:qo + 128],
                            rhs=vv_rhs(oh, kb),
                            start=first,
                            stop=last,
                        )
                    nc.tensor.matmul(
                        lps, lhsT=pt[:, qo:qo + 128], rhs=ones,
                        start=first, stop=last,
                    )
                rec = smlp.tile([128, 1], F32, tag="rec")
                nc.vector.reciprocal(rec, lps[:, 0:1])
                ot = otp.tile([128, D], F32, tag="ot")
                nc.vector.tensor_scalar_mul(ot[:, 0:512], avs[0], rec)
                nc.scalar.activation(ot[:, 512:1024], avs[1], COPY, scale=rec)
                nc.scalar.dma_start(
                    out=out[ci * 128:(ci + 1) * 128, 0:512], in_=ot[:, 0:512]
                )
                nc.sync.dma_start(
                    out=out[ci * 128:(ci + 1) * 128, 512:1024],
                    in_=ot[:, 512:1024],
                )
    nc.compile()
    return nc


def _pair_blocks(sub):
    """(lo_i, hi_i) query-block ids for pairs i=0..3."""
    return [(2 * i + sub, 15 - 2 * i - sub) for i in range(4)]


def _query_cols(sub):
    """qt/xTq column order: [lo0 lo1 lo2 lo3 hi0 hi1 hi2 hi3] x 128."""
    pb = _pair_blocks(sub)
    cols = [np.arange(lo * 128, lo * 128 + 128) for lo, _ in pb]
    cols += [np.arange(hi * 128, hi * 128 + 128) for _, hi in pb]
    return np.concatenate(cols)


def _chain_blocks(sub):
    """Output row order: chains lo3..lo0 then hi0..hi3."""
    pb = _pair_blocks(sub)
    return [pb[i][0] for i in (3, 2, 1, 0)] + [pb[i][1] for i in range(4)]


def _masks(sub):
    """masks_lo / masks_hi [128, 8, 512]: slot kb (lo) / kb-8 (hi)."""
    p = np.arange(128)[:, None]
    j = np.arange(128)[None, :]
    pb = _pair_blocks(sub)
    mlo = np.ones((8, 128, 512), np.float32)
    mhi = np.ones((8, 128, 512), np.float32)
    for kb in range(8):
        i0 = _i0(kb)
        for c, i in enumerate(range(i0, 4)):
            qb = pb[i][0]
            mlo[kb, :, c * 128:(c + 1) * 128] = (kb * 128 + p <= qb * 128 + j)
    for kb in range(8, 16):
        for i in range(_h(kb)):
            qb = pb[i][1]
            mhi[kb - 8, :, i * 128:(i + 1) * 128] = (
                kb * 128 + p <= qb * 128 + j
            )
    return (
        np.ascontiguousarray(mlo.transpose(1, 0, 2)),
        np.ascontiguousarray(mhi.transpose(1, 0, 2)),
    )


def _pmaj(a):
    """[dc*128, cols] -> partition-major [128, dc, cols]."""
    d, cols = a.shape
    return np.ascontiguousarray(a.reshape(d // 128, 128, cols).transpose(1, 0, 2))


def kernel(x, Wq, Wk, Wv, _trace=False):
    if "nc" not in _cache:
        _cache["nc"] = build_nc()
    nc = _cache["nc"]

    x = np.asarray(x, dtype=np.float32)
    # q' = 32q, k' = 32k: x32 into Wq (net of the folded 1/sqrt(d)) and Wk;
    # scores then carry 32*32*32 = 2^15, removed by EXP_SCALE.
    WqTs = _pmaj((np.asarray(Wq, np.float32).T * np.float32(32.0)).astype(BFNP))
    WkTs = (np.asarray(Wk, np.float32).T * np.float32(32.0)).astype(BFNP)
    WvT = np.asarray(Wv, np.float32).T.astype(BFNP)

    in_maps = []
    for c in range(8):
        b, sub = c // 2, c % 2
        xTb = x[b].T.astype(BFNP)                      # [1024, 2048]
        xTp = _pmaj(xTb)                               # [128, 8, 2048]
        xT4 = np.ascontiguousarray(
            xTp.reshape(128, 2, 4, 4, 512).transpose(0, 3, 1, 2, 4)
        )                                              # [128, sc, h, dc%4, 512]
        wkh = _pmaj(WkTs[:, sub * 512:(sub + 1) * 512])  # [128, 8, 512]
        wk4 = np.ascontiguousarray(
            wkh.reshape(128, DC, 4, 128).transpose(0, 2, 1, 3)
        )                                              # [128, 4, 8, 128]
        wvh = _pmaj(WvT[:, sub * 512:(sub + 1) * 512])   # [128, 8, 512]
        wv2 = np.ascontiguousarray(wvh.reshape(128, 2, 4, 512))
        mlo, mhi = _masks(sub)
        in_maps.append(
            {
                "xT": xT4,
                "xTq": _pmaj(np.ascontiguousarray(xTb[:, _query_cols(sub)])),
                "WqT": WqTs,
                "WkTh": wk4,
                "WvTh": wv2,
                "masks_lo": mlo.astype(BFNP),
                "masks_hi": mhi.astype(BFNP),
            }
        )

    res = run_bass_kernel_spmd(
        nc, in_maps, core_ids=list(range(8)), trace=_trace
    )
    full = np.empty((B, S, D), np.float32)
    for c in range(8):
        b, sub = c // 2, c % 2
        for pos, qb in enumerate(_chain_blocks(sub)):
            full[b, qb * 128:(qb + 1) * 128] = (
                res.results[c]["out"][pos * 128:(pos + 1) * 128]
            )
    if _trace:
        _cache["last_result"] = res
    return full


# revision 13
# speedup vs baseline: 1.2321x; 1.0177x over previous
"""Causal single-head attention (B=4, S=2048, d=1024) on 8 TRN2 NeuronCores.

Sharding: core c -> batch b = c//2, subset s = c%2. Per batch the 16
query blocks (128 rows) are assigned in balanced causal pairs: core
(b,s) owns pairs (lo_i, hi_i) = (2i+s, 15-2i-s), i=0..3, giving every
core 68 true causal score tiles (padded to a uniform 72). Every core
runs the identical instruction stream; causal boundaries come from
per-core 0/1 mask tiles supplied as input data.

K/V projections are tensor-parallel within each core pair: core (b,s)
computes the d_out-half s of kT (fp8, x32-scaled) and v (bf16) for the
whole batch; halves are exchanged with pairwise AllGathers
([[0,1],[2,3],[4,5],[6,7]]). The gathers are ~25us latency-bound and
serialize on the CC core, so K's gather goes first (Phase A needs it
~20us before Phase B needs V's).

Precision: projections and AV run in bf16 (fp32 PSUM). Scores run in
fp8e4m3 DoubleRow (256-deep contraction per pass = 2x bf16 FLOPs):
host folds x32 into Wq and Wk so q' = 32q, k' = 32k sit in the fp8
sweet spot; the combined 2^15 scale is removed inside the EXP
activation (exp(s' / 32768)).

qt8 column layout is [lo0 lo1 lo2 lo3 hi0 hi1 hi2 hi3] (128 cols per
block). At key block kb the live lo blocks are the suffix i >= kb//2
(kb < 8) and the live hi blocks are the prefix i < h(kb); both are
contiguous column ranges, so Phase A needs only TWO wide DR matmuls
per (kb, dc-pair) into two PSUM tiles. Phase B chains slice columns
out of the per-kb exp tiles.
"""
import sys

sys.path.insert(0, "/opt/trn_rl_repo")

import ml_dtypes
import numpy as np

import concourse.bass as bass  # noqa: F401
import concourse.mybir as mybir
import concourse.tile as tile
from concourse import bacc
from concourse.bass_utils import run_bass_kernel_spmd

B, S, D = 4, 2048, 1024
DC = D // 128          # 8 contraction chunks
F32 = mybir.dt.float32
BF = mybir.dt.bfloat16
E4 = mybir.dt.float8e4
E4NP = ml_dtypes.float8_e4m3
BFNP = ml_dtypes.bfloat16
DR = mybir.MatmulPerfMode.DoubleRow
EXP = mybir.ActivationFunctionType.Exp
COPY = mybir.ActivationFunctionType.Copy
GROUPS = [[0, 1], [2, 3], [4, 5], [6, 7]]
EXP_SCALE = 1.0 / 32768.0   # q,k both carry x32; scores carry x1024*32


def _i0(kb):
    """First live lo block at key block kb (suffix i0..3), kb < 8."""
    return kb // 2


def _h(kb):
    """Number of live hi blocks at key block kb (prefix 0..h-1)."""
    return min(4, (17 - kb) // 2)


_cache = {}


def build_nc():
    nc = bacc.Bacc("TRN2", target_bir_lowering=False, debug=False, num_devices=8)
    # inputs, partition-major & contiguous per planned DMA
    xT = nc.dram_tensor("xT", [128, 4, 2, 4, 512], BF, kind="ExternalInput")
    xTq = nc.dram_tensor("xTq", [128, DC, 1024], BF, kind="ExternalInput")
    WqT = nc.dram_tensor("WqT", [128, DC, D], BF, kind="ExternalInput")
    WkTh = nc.dram_tensor("WkTh", [128, 4, DC, 128], BF, kind="ExternalInput")
    WvTh = nc.dram_tensor("WvTh", [128, 2, 4, 512], BF, kind="ExternalInput")
    masks_lo = nc.dram_tensor("masks_lo", [128, 8, 512], BF, kind="ExternalInput")
    masks_hi = nc.dram_tensor("masks_hi", [128, 8, 512], BF, kind="ExternalInput")
    out = nc.dram_tensor("out", [1024, D], F32, kind="ExternalOutput")
    # collective buffers
    kg_in = [
        nc.dram_tensor(f"kg_in{h}", [128, 4, 1024], E4) for h in range(2)
    ]
    kg_out = [
        nc.dram_tensor(f"kg_out{h}", [2, 128, 4, 1024], E4) for h in range(2)
    ]
    vg_in = [
        nc.dram_tensor(f"vg_in{h}", [128, 8, 512], BF) for h in range(2)
    ]
    vg_out = [
        nc.dram_tensor(f"vg_out{h}", [2, 128, 8, 512], BF) for h in range(2)
    ]

    with tile.TileContext(nc) as tc:
        with (
            tc.tile_pool(name="w", bufs=1) as wp,
            tc.tile_pool(name="xs", bufs=1) as xsp,
            tc.tile_pool(name="per", bufs=1) as per,
            tc.tile_pool(name="pt", bufs=1) as ptp,
            tc.tile_pool(name="ot", bufs=2) as otp,
            tc.tile_pool(name="sml", bufs=4) as smlp,
            tc.tile_pool(name="mix", bufs=4, space="PSUM") as mixp,
            tc.tile_pool(name="psav", bufs=4, space="PSUM") as psavp,
        ):
            # ---------------- consts + persistent ----------------
            zeros_f = per.tile([128, 2], F32)
            ones = per.tile([128, 2], BF)
            nc.vector.memset(zeros_f, 0.0)
            # exp(0)=1 -> also preloads the ACT exp table long before A
            nc.scalar.activation(ones, zeros_f, EXP)

            # fine-grained head DMAs: first K-proj group needs wk_0 + xs_0_0
            wk = [wp.tile([128, DC, 128], BF, name=f"wk_{o}") for o in range(4)]
            xs = [
                [xsp.tile([128, 4, 512], BF, name=f"xs_{sc}_{h}") for h in range(2)]
                for sc in range(4)
            ]
            nc.sync.dma_start(out=wk[0], in_=WkTh[:, 0])
            nc.sync.dma_start(out=xs[0][0], in_=xT[:, 0, 0])
            nc.sync.dma_start(out=xs[0][1], in_=xT[:, 0, 1])
            for o in range(1, 4):
                nc.sync.dma_start(out=wk[o], in_=WkTh[:, o])
            for sc in range(1, 4):
                for h in range(2):
                    nc.sync.dma_start(out=xs[sc][h], in_=xT[:, sc, h])
            wv = [wp.tile([128, 4, 512], BF, name=f"wv_{h}") for h in range(2)]
            for h in range(2):
                nc.sync.dma_start(out=wv[h], in_=WvTh[:, h])
            wq = wp.tile([128, DC, D], BF)
            xq = wp.tile([128, DC, 1024], BF)
            nc.sync.dma_start(out=wq, in_=WqT[:])
            nc.sync.dma_start(out=xq, in_=xTq[:])
            maskt_lo = per.tile([128, 8, 512], BF)
            maskt_hi = per.tile([128, 8, 512], BF)
            nc.sync.dma_start(out=maskt_lo, in_=masks_lo[:])
            nc.sync.dma_start(out=maskt_hi, in_=masks_hi[:])

            # -------- P1: K half-projection (fp8 out, x32 folded) --------
            # key-halves gathered separately: each 1MB collective starts at
            # the producing half's completion, pipelining the CC core.
            kg_sb = [
                per.tile([128, 4, 1024], E4, name=f"kg_sb{h}") for h in range(2)
            ]
            for sc in range(4):
                for ocl in range(4):
                    ps = mixp.tile([128, 512], F32, tag="mix")
                    for dc in range(DC):
                        nc.tensor.matmul(
                            ps,
                            lhsT=wk[ocl][:, dc, :],
                            rhs=xs[sc][dc // 4][:, dc % 4, :],
                            start=(dc == 0),
                            stop=(dc == DC - 1),
                        )
                    nc.vector.tensor_copy(
                        kg_sb[sc // 2][:, ocl, (sc % 2) * 512:(sc % 2) * 512 + 512],
                        ps,
                    )
                if sc % 2 == 1:
                    h = sc // 2
                    nc.scalar.dma_start(out=kg_in[h][:], in_=kg_sb[h])
                    nc.gpsimd.collective_compute(
                        "AllGather",
                        mybir.AluOpType.bypass,
                        replica_groups=GROUPS,
                        ins=[kg_in[h][:]],
                        outs=[kg_out[h][:]],
                    )

            # -------- P2: V half-projection (bf16) --------
            # key-halves gathered separately: the first 1MB collective can
            # start at P2's midpoint, right after AG_K frees the CC core.
            vg_sb = [
                per.tile([128, 8, 512], BF, name=f"vg_sb{h}") for h in range(2)
            ]
            for sc in range(4):
                for sb in range(4):
                    ps = mixp.tile([128, 512], F32, tag="mix", name=f"ps2_{sc}_{sb}")
                    for dc in range(DC):
                        nc.tensor.matmul(
                            ps,
                            lhsT=xs[sc][dc // 4][:, dc % 4, sb * 128:(sb + 1) * 128],
                            rhs=wv[dc // 4][:, dc % 4, :],
                            start=(dc == 0),
                            stop=(dc == DC - 1),
                        )
                    nc.vector.tensor_copy(
                        vg_sb[sc // 2][:, (sc % 2) * 4 + sb, :], ps
                    )
                if sc % 2 == 1:
                    h = sc // 2
                    nc.scalar.dma_start(out=vg_in[h][:], in_=vg_sb[h])
                    nc.gpsimd.collective_compute(
                        "AllGather",
                        mybir.AluOpType.bypass,
                        replica_groups=GROUPS,
                        ins=[vg_in[h][:]],
                        outs=[vg_out[h][:]],
                    )

            # -------- load gathered kt8 (key-halves so A starts early) ---
            kt8 = [wp.tile([128, DC, 1024], E4, name=f"kt8_{ch}") for ch in range(2)]
            for ch in range(2):
                for r in range(2):
                    nc.sync.dma_start(
                        out=kt8[ch][:, 4 * r:4 * r + 4, :],
                        in_=kg_out[ch][r],
                    )

            # -------- P0: Q projection -> qt8 (fp8, overlaps gathers) ----
            qt8 = per.tile([128, DC, 1024], E4)
            for oc in range(8):
                pss = [
                    mixp.tile([128, 512], F32, tag="mix", name=f"ps0_{oc}_{i}")
                    for i in range(2)
                ]
                for dc in range(DC):
                    for sc in range(2):
                        nc.tensor.matmul(
                            pss[sc],
                            lhsT=wq[:, dc, oc * 128:(oc + 1) * 128],
                            rhs=xq[:, dc, sc * 512:(sc + 1) * 512],
                            start=(dc == 0),
                            stop=(dc == DC - 1),
                        )
                for sc in range(2):
                    nc.vector.tensor_copy(
                        qt8[:, oc, sc * 512:(sc + 1) * 512], pss[sc]
                    )

            # -------- load gathered vv (key-halves; lo chains only need a)
            vva = per.tile([128, 2, 8, 512], BF)
            vvb = per.tile([128, 2, 8, 512], BF)
            for r in range(2):
                nc.sync.dma_start(out=vva[:, r], in_=vg_out[0][r])
            for r in range(2):
                for g in range(2):
                    nc.sync.dma_start(
                        out=vvb[:, r, 4 * g:4 * g + 4],
                        in_=vg_out[1][r][:, 4 * g:4 * g + 4, :],
                    )

            def vv_rhs(oh, kb):
                t = vva if kb < 8 else vvb
                return t[:, oh, kb % 8, :]

            # ------- Phase A: scoresT + exp + mask, two wide tiles per kb
            pt_lo = {}
            pt_hi = {}
            for kb in range(16):
                ktc = kt8[kb // 8]
                kcol = (kb % 8) * 128
                h = _h(kb)
                tiles = [("hi", 512, 128 * h)]
                if kb < 8:
                    i0 = _i0(kb)
                    tiles.append(("lo", 128 * i0, 128 * (4 - i0)))
                pss = {}
                for kind, qo, qw in tiles:
                    pss[kind] = mixp.tile(
                        [128, 512], F32, tag="mix", name=f"psA_{kb}_{kind}"
                    )
                for j in range(4):
                    for kind, qo, qw in tiles:
                        nc.tensor.matmul(
                            pss[kind][:, 0:qw],
                            lhsT=ktc[:, 2 * j:2 * j + 2, kcol:kcol + 128],
                            rhs=qt8[:, 2 * j:2 * j + 2, qo:qo + qw],
                            start=(j == 0),
                            stop=(j == 3),
                            perf_mode=DR,
                        )
                for kind, qo, qw in tiles:
                    pt = ptp.tile([128, qw], BF, name=f"pt_{kind}_{kb}")
                    nc.scalar.activation(pt, pss[kind][:, 0:qw], EXP,
                                         scale=EXP_SCALE)
                    if kind == "lo":
                        nc.vector.tensor_mul(pt, pt, maskt_lo[:, kb, 0:qw])
                        pt_lo[kb] = pt
                    else:
                        if kb >= 8:
                            nc.vector.tensor_mul(
                                pt, pt, maskt_hi[:, kb - 8, 0:qw]
                            )
                        pt_hi[kb] = pt

            # ---------------- Phase B: chains ----------------
            # lo chains first (need only vva), then hi chains.
            chains = []
            for i in (3, 2, 1, 0):
                chains.append([
                    (pt_lo[kb], 128 * (i - _i0(kb)), kb)
                    for kb in range(0, 2 * i + 2)
                ])
            for i in range(4):
                chains.append([
                    (pt_hi[kb], 128 * i, kb) for kb in range(0, 16 - 2 * i)
                ])

            for ci, tiles in enumerate(chains):
                avs = [
                    psavp.tile([128, 512], F32, tag="psav", name=f"av_{ci}_{oh}")
                    for oh in range(2)
                ]
                lps = psavp.tile([128, 2], F32, tag="psav", name=f"l_{ci}")
                n = len(tiles)
                for idx, (pt, qo, kb) in enumerate(tiles):
                    first, last = idx == 0, idx == n - 1
                    for oh in range(2):
                        nc.tensor.matmul(
                            avs[oh],
                            lhsT=pt[:, qo:qo + 128],
                            rhs=vv_rhs(oh, kb),
                            start=first,
                            stop=last,
                        )
                    nc.tensor.matmul(
                        lps, lhsT=pt[:, qo:qo + 128], rhs=ones,
                        start=first, stop=last,
                    )
                rec = smlp.tile([128, 1], F32, tag="rec")
                nc.vector.reciprocal(rec, lps[:, 0:1])
                ot = otp.tile([128, D], F32, tag="ot")
                nc.vector.tensor_scalar_mul(ot[:, 0:512], avs[0], rec)
                nc.scalar.activation(ot[:, 512:1024], avs[1], COPY, scale=rec)
                nc.scalar.dma_start(
                    out=out[ci * 128:(ci + 1) * 128, 0:512], in_=ot[:, 0:512]
                )
                nc.sync.dma_start(
                    out=out[ci * 128:(ci + 1) * 128, 512:1024],
                    in_=ot[:, 512:1024],
                )
    nc.compile()
    return nc


def _pair_blocks(sub):
    """(lo_i, hi_i) query-block ids for pairs i=0..3."""
    return [(2 * i + sub, 15 - 2 * i - sub) for i in range(4)]


def _query_cols(sub):
    """qt/xTq column order: [lo0 lo1 lo2 lo3 hi0 hi1 hi2 hi3] x 128."""
    pb = _pair_blocks(sub)
    cols = [np.arange(lo * 128, lo * 128 + 128) for lo, _ in pb]
    cols += [np.arange(hi * 128, hi * 128 + 128) for _, hi in pb]
    return np.concatenate(cols)


def _chain_blocks(sub):
    """Output row order: chains lo3..lo0 then hi0..hi3."""
    pb = _pair_blocks(sub)
    return [pb[i][0] for i in (3, 2, 1, 0)] + [pb[i][1] for i in range(4)]


def _masks(sub):
    """masks_lo / masks_hi [128, 8, 512]: slot kb (lo) / kb-8 (hi)."""
    p = np.arange(128)[:, None]
    j = np.arange(128)[None, :]
    pb = _pair_blocks(sub)
    mlo = np.ones((8, 128, 512), np.float32)
    mhi = np.ones((8, 128, 512), np.float32)
    for kb in range(8):
        i0 = _i0(kb)
        for c, i in enumerate(range(i0, 4)):
            qb = pb[i][0]
            mlo[kb, :, c * 128:(c + 1) * 128] = (kb * 128 + p <= qb * 128 + j)
    for kb in range(8, 16):
        for i in range(_h(kb)):
            qb = pb[i][1]
            mhi[kb - 8, :, i * 128:(i + 1) * 128] = (
                kb * 128 + p <= qb * 128 + j
            )
    return (
        np.ascontiguousarray(mlo.transpose(1, 0, 2)),
        np.ascontiguousarray(mhi.transpose(1, 0, 2)),
    )


def _pmaj(a):
    """[dc*128, cols] -> partition-major [128, dc, cols]."""
    d, cols = a.shape
    return np.ascontiguousarray(a.reshape(d // 128, 128, cols).transpose(1, 0, 2))


def kernel(x, Wq, Wk, Wv, _trace=False):
    if "nc" not in _cache:
        _cache["nc"] = build_nc()
    nc = _cache["nc"]

    x = np.asarray(x, dtype=np.float32)
    # q' = 32q, k' = 32k: x32 into Wq (net of the folded 1/sqrt(d)) and Wk;
    # scores then carry 32*32*32 = 2^15, removed by EXP_SCALE.
    WqTs = _pmaj((np.asarray(Wq, np.float32).T * np.float32(32.0)).astype(BFNP))
    WkTs = (np.asarray(Wk, np.float32).T * np.float32(32.0)).astype(BFNP)
    WvT = np.asarray(Wv, np.float32).T.astype(BFNP)

    in_maps = []
    for c in range(8):
        b, sub = c // 2, c % 2
        xTb = x[b].T.astype(BFNP)                      # [1024, 2048]
        xTp = _pmaj(xTb)                               # [128, 8, 2048]
        xT4 = np.ascontiguousarray(
            xTp.reshape(128, 2, 4, 4, 512).transpose(0, 3, 1, 2, 4)
        )                                              # [128, sc, h, dc%4, 512]
        wkh = _pmaj(WkTs[:, sub * 512:(sub + 1) * 512])  # [128, 8, 512]
        wk4 = np.ascontiguousarray(
            wkh.reshape(128, DC, 4, 128).transpose(0, 2, 1, 3)
        )                                              # [128, 4, 8, 128]
        wvh = _pmaj(WvT[:, sub * 512:(sub + 1) * 512])   # [128, 8, 512]
        wv2 = np.ascontiguousarray(wvh.reshape(128, 2, 4, 512))
        mlo, mhi = _masks(sub)
        in_maps.append(
            {
                "xT": xT4,
                "xTq": _pmaj(np.ascontiguousarray(xTb[:, _query_cols(sub)])),
                "WqT": WqTs,
                "WkTh": wk4,
                "WvTh": wv2,
                "masks_lo": mlo.astype(BFNP),
                "masks_hi": mhi.astype(BFNP),
            }
        )

    res = run_bass_kernel_spmd(
        nc, in_maps, core_ids=list(range(8)), trace=_trace
    )
    full = np.empty((B, S, D), np.float32)
    for c in range(8):
        b, sub = c // 2, c % 2
        for pos, qb in enumerate(_chain_blocks(sub)):
            full[b, qb * 128:(qb + 1) * 128] = (
                res.results[c]["out"][pos * 128:(pos + 1) * 128]
            )
    if _trace:
        _cache["last_result"] = res
    return full
